# revision 29
# baseline (speedup 1.0000x reference)
"""Local-window multi-head attention (window=33) for Trainium2, 8-core SPMD.

Sharding: data-parallel over batch (B=8 -> 1 batch per core). Weights
replicated. Per core: QKV projections (fp16 matmuls, f32 accum), banded
local attention via transposed-score blocks of 96 queries x 128 keys,
output projection, all fused in one Bass/Tile kernel.

Layout notes:
  - x fed transposed (host prep): xT [512, S] fp16 so projections
    contract din on partitions.
  - q^T, k^T kept [dout, pos] fp16 (head h = rows 64*(h%2) of chunk
    h//2). MM1 runs K=64 at partition base 0/64 per head.
  - scores computed transposed: S^T[kpos, q] = k_h^T.T @ q_h^T, so the
    key-side mask/range penalty is a per-partition bias of the exp on
    ScalarE, and MM2 (P^T as lhsT) gives attn natural [q, d] with the
    softmax denominator from a ones-column appended to v.
  - k-bias dropped entirely: (q+bq)or(k+bk) scores differ by a
    per-query constant, which softmax cancels. Output bias
    (Wo@bv + bo) added on host.
  - v projected per 96-query block directly into [kpos 128, head, 66]
    ring tiles (cols 64:66 = ones, memset once), so MM2 needs no
    K-splits.
"""
import contextlib
import os
import sys
sys.path.insert(0, "/opt/trn_rl_repo")
import numpy as np

B, S, D, H, HD = 8, 4096, 512, 8, 64
WIN, HALF = 33, 16
QB = 96
NB = (S + QB - 1) // QB          # 43 blocks (42 full + 64)
CPB = int(os.environ.get("BK_CPB", "5"))  # blocks per chunk
QW = QB * CPB                    # 480
KW = QW + 2 * HALF               # 512
NEG = -1e9

_CHUNKS = [list(range(c * CPB, min(NB, (c + 1) * CPB))) for c in range((NB + CPB - 1) // CPB)]

_NCS = {}
REPS = int(os.environ.get("BASS_KERNEL_REPS", "1"))
USE_BF16 = os.environ.get("BASS_KERNEL_BF16", "1") == "1"
BK_TR = int(os.environ.get("BK_TR", "1"))        # tr_ps bufs
BK_MM2 = int(os.environ.get("BK_MM2", "2"))      # mm2_ps bufs
BK_ST = int(os.environ.get("BK_ST", "2"))        # st_ps bufs
BK_PROJ = int(os.environ.get("BK_PROJ", "2"))    # proj_ps bufs (op shares)
BK_STP = int(os.environ.get("BK_STP", "2"))
BK_QKP = int(os.environ.get("BK_QKP", "2"))
BK_PTP = int(os.environ.get("BK_PTP", "3"))
BK_SMP = int(os.environ.get("BK_SMP", "3"))
BK_ANY = os.environ.get("BK_ANY", "1") == "1"    # nc.any copies


def _qw(j):
    return min(QB, S - QB * j)


def _build(reps=None):
    reps = REPS if reps is None else reps
    import concourse.bacc as bacc
    import concourse.mybir as mybir
    from concourse.tile import TileContext

    F32 = mybir.dt.float32
    BF16 = mybir.dt.bfloat16
    F16 = mybir.dt.float16
    LOW = BF16                    # exp/prob/v path (range needs bf16)
    XDT = F16                     # x / W / q / k path (precision)
    EXP = mybir.ActivationFunctionType.Exp
    CPYF = mybir.ActivationFunctionType.Identity
    MULT = mybir.AluOpType.mult

    nc = bacc.Bacc(None, target_bir_lowering=False)

    SP = S + 64                   # 16 left pad + 48 right pad
    BK_RES = os.environ.get("BK_RES", "0") == "1"
    BK_XALL = os.environ.get("BK_XALL", "0") == "1" or BK_RES
    if BK_XALL:
        xallT = nc.dram_tensor("xallT", [3 * D, SP], XDT, kind="ExternalInput")
        xqT = xkvT = None
    else:
        xqT = nc.dram_tensor("xqT", [D, S], XDT, kind="ExternalInput")
        xkvT = nc.dram_tensor("xkvT", [2 * D, SP], XDT, kind="ExternalInput")
    wqkvT = nc.dram_tensor("wqkvT", [3 * D, D], XDT, kind="ExternalInput")
    woT = nc.dram_tensor("woT", [D, D], LOW, kind="ExternalInput")
    cst32_d = nc.dram_tensor("cst32", [128, 4 + NB], F32, kind="ExternalInput")
    cst16_d = nc.dram_tensor("cst16", [128, H * QB + 128], LOW,
                             kind="ExternalInput")
    out_d = nc.dram_tensor("out", [S, D], F16, kind="ExternalOutput")

    def r4(t):  # [512, N] dram -> [128, 4, N] view
        return t[:, :].rearrange("(c p) n -> p c n", p=128)

    with TileContext(nc) as tc:
        with tc.tile_pool(name="const", bufs=1) as cp, \
             tc.tile_pool(name="stage", bufs=BK_STP) as stp, \
             tc.tile_pool(name="qk", bufs=BK_QKP) as qkp, \
             tc.tile_pool(name="pt", bufs=BK_PTP) as ptp, \
             tc.tile_pool(name="small", bufs=BK_SMP) as smp, \
             tc.tile_pool(name="outp", bufs=3) as outp, \
             tc.tile_pool(name="proj_ps", bufs=BK_PROJ, space="PSUM") as proj_ps, \
             tc.tile_pool(name="st_ps", bufs=BK_ST, space="PSUM") as st_ps, \
             tc.tile_pool(name="mm2_ps", bufs=BK_MM2, space="PSUM") as mm2_ps, \
             tc.tile_pool(name="tr_ps", bufs=BK_TR, space="PSUM") as tr_ps, \
             tc.tile_pool(name="op_ps", bufs=1, space="PSUM") as op_sep:
            op_ps = proj_ps if os.environ.get("BK_OPSH", "0") == "1" else op_sep

            # ---- constants ----
            w_all = cp.tile([128, 12, D], XDT, name="w_all")
            wo_sb = cp.tile([128, 4, D], LOW, name="wo_sb")
            nc.sync.dma_start(
                w_all[:], wqkvT[:, :].rearrange("(c p) n -> p c n", p=128))
            nc.sync.dma_start(wo_sb[:], r4(woT))
            wq_sb = w_all[:, 0:4, :].rearrange("p c n -> p c n")
            wk_sb = w_all[:, 4:8, :].rearrange("p c n -> p c n")
            wv_sb = w_all[:, 8:12, :].rearrange("p c n -> p c n")
            cst32 = cp.tile([128, 4 + NB], F32, name="cst32_sb")
            cst16 = cp.tile([128, H * QB + 128], LOW, name="cst16_sb")
            nc.sync.dma_start(cst32[:], cst32_d[:, :])
            nc.sync.dma_start(cst16[:], cst16_d[:, :])
            bqc = cst32[:, 0:4]
            pen = cst32[:, 4:4 + NB]
            band = cst16[:, 0:H * QB].rearrange("p (h q) -> p h q", q=QB)
            iden = cst16[:, H * QB:H * QB + 128]

            # persistent v ring: [kpos 128, head, 64+2] with ones cols
            # memset once; per-block copies only touch cols 0:64
            NVR = CPB + 3
            vring = []
            for i in range(NVR):
                vt = cp.tile([128, H, 66], LOW, name=f"vt{i}")
                nc.gpsimd.memset(vt[:, :, 64:66], 1.0)
                vring.append(vt)

            # persistent double-buffered qZ: per-head q^T with the other
            # co-projected head's partition half zeroed, so MM1 runs as a
            # full-K=128 matmul at partition base 0 (base-64 operands fault
            # at runtime)
            BK_QC = os.environ.get("BK_QC", "0") == "1"
            qZ_bufs = []
            if not BK_QC:
                for bi in range(2):
                    qz = cp.tile([128, H, QW], XDT, name=f"qZ{bi}")
                    nc.gpsimd.memset(qz[0:64, 1:H:2, :], 0.0)
                    nc.gpsimd.memset(qz[64:128, 0:H:2, :], 0.0)
                    qZ_bufs.append(qz)

            loop_cm = (tc.For_i(0, reps, 1) if reps > 1
                       else contextlib.nullcontext())
            with loop_cm:
                xres = None
                if BK_RES:
                    xres = cp.tile([128, 12, SP], XDT, name="xres")
                    av = xallT[:, :].rearrange("(t c p) n -> p t c n",
                                               p=128, c=4)
                    avf = av.rearrange("p t c n -> p (t c) n")
                    nc.sync.dma_start(xres[:, :, 0:1024], avf[:, :, 0:1024])
                    nc.sync.dma_start(xres[:, :, 1024:SP], avf[:, :, 1024:SP])
                osc2 = osc2_lo = None
                xq2 = xkv2 = None
                INB2 = os.environ.get("BK_INB2", "1") == "1"
                for ci, blocks in enumerate(_CHUNKS):
                    j0, j1 = blocks[0], blocks[-1]
                    q_lo = QB * j0
                    q_hi = min(S, QB * (j1 + 1))
                    qwid = q_hi - q_lo                       # 480 or 256
                    win_lo = QB * j0 - HALF                  # may be < 0
                    win_hi = QB * j1 + 112                   # may be > S
                    kwid = win_hi - win_lo                   # 512 or 320
                    dlo, dhi = max(0, win_lo), min(S, win_hi)

                    # ---- stage x^T slices (q/k/v fused + host-padded) ----
                    if BK_RES:
                        base = HALF + win_lo

                        class _QV:
                            def __getitem__(self, idx):
                                p, k, sl = idx
                                return xres[p, k,
                                            HALF + q_lo + sl.start:
                                            HALF + q_lo + sl.stop]
                        xq_st_v = _QV()

                        class _KV:
                            def __getitem__(self, idx):
                                p, t, k, sl = idx
                                return xres[p, 4 * (1 + t) + k,
                                            base + sl.start:base + sl.stop]
                        xkv_st = _KV()
                    elif BK_XALL:
                        xa_st = stp.tile([128, 3, 4, KW], XDT, tag="xa_st",
                                         name="xa_st")
                        av = xallT[:, :].rearrange("(t c p) n -> p t c n",
                                                   p=128, c=4)
                        nc.sync.dma_start(xa_st[:, :, :, :kwid],
                                          av[:, :, :, HALF + win_lo:HALF + win_hi])

                        class _QV:
                            def __getitem__(self, idx):
                                p, k, sl = idx
                                return xa_st[p, 0, k,
                                             sl.start + HALF:sl.stop + HALF]
                        xq_st_v = _QV()
                        xkv_st = xa_st[:, 1:3, :, :]
                    elif INB2 and qwid == 480 and (ci % 2 == 0 and
                                                   ci + 1 < len(_CHUNKS) and
                                                   len(_CHUNKS[ci + 1]) == CPB
                                                   or ci % 2 == 1):
                        # paired staging: even ci loads both chunks' windows
                        if ci % 2 == 0:
                            xq2 = stp.tile([128, 4, 2 * QW], XDT, tag="xq_st",
                                           name="xq_st")
                            xkv2 = stp.tile([128, 2, 4, QW + KW], XDT,
                                            tag="xkv_st", name="xkv_st")
                            nc.sync.dma_start(
                                xq2[:, :, :], r4(xqT)[:, :, q_lo:q_lo + 2 * QW])
                            kv = xkvT[:, :].rearrange("(t c p) n -> p t c n",
                                                      p=128, c=4)
                            nc.sync.dma_start(
                                xkv2[:, :, :, :],
                                kv[:, :, :, HALF + win_lo:
                                   HALF + win_lo + QW + KW])
                        co = QW * (ci % 2)
                        xq_st = xq2[:, :, co:co + QW]
                        xkv_st = xkv2[:, :, :, co:co + KW]
                    else:
                        xq_st = stp.tile([128, 4, QW], XDT, tag="xq_st",
                                         name="xq_st")
                        xkv_st = stp.tile([128, 2, 4, KW], XDT, tag="xkv_st",
                                          name="xkv_st")
                        nc.sync.dma_start(xq_st[:, :, :qwid],
                                          r4(xqT)[:, :, q_lo:q_hi])
                        kv = xkvT[:, :].rearrange("(t c p) n -> p t c n",
                                                  p=128, c=4)
                        nc.sync.dma_start(xkv_st[:, :, :, :kwid],
                                          kv[:, :, :, HALF + win_lo:HALF + win_hi])

                    # ---- q^T / k^T projections (<=512-wide splits) ----
                    qZ = None if BK_QC else qZ_bufs[ci % 2]
                    qC = (qkp.tile([128, 4, QW], XDT, tag="qC", name="qC")
                          if BK_QC else None)
                    kT = qkp.tile([128, 4, KW], XDT, tag="kT", name="kT")

                    def halves(w):
                        return [(0, w)] if w <= 512 else [(0, w // 2),
                                                          (w // 2, w - w // 2)]

                    for dc in range(4):
                        for off, nw in halves(qwid):
                            ps = proj_ps.tile([128, 512], F32, tag="proj",
                                              name="qps")
                            for k in range(4):
                                rhs = (xq_st_v[slice(None), k,
                                               slice(off, off + nw)]
                                       if BK_XALL else
                                       xq_st[:, k, off:off + nw])
                                nc.tensor.matmul(ps[:, :nw],
                                                 wq_sb[:, k, 128 * dc:128 * dc + 128],
                                                 rhs,
                                                 start=(k == 0), stop=(k == 3))
                            sl = slice(off, off + nw)
                            if BK_QC:
                                nc.vector.tensor_scalar_add(
                                    qC[:, dc, sl], ps[:, :nw],
                                    bqc[:, dc:dc + 1])
                            elif dc < int(os.environ.get("BK_QSPLIT", "1")):
                                nc.scalar.activation(
                                    qZ[0:64, 2 * dc, sl], ps[0:64, :nw],
                                    CPYF, bias=bqc[0:64, dc:dc + 1], scale=1.0)
                                nc.scalar.activation(
                                    qZ[64:128, 2 * dc + 1, sl], ps[64:128, :nw],
                                    CPYF, bias=bqc[64:128, dc:dc + 1], scale=1.0)
                            else:
                                nc.vector.tensor_scalar_add(
                                    qZ[0:64, 2 * dc, sl], ps[0:64, :nw],
                                    bqc[0:64, dc:dc + 1])
                                nc.vector.tensor_scalar_add(
                                    qZ[64:128, 2 * dc + 1, sl], ps[64:128, :nw],
                                    bqc[64:128, dc:dc + 1])
                    for dc in range(4):
                        for off, nw in halves(kwid):
                            ps = proj_ps.tile([128, 512], F32, tag="proj",
                                              name="kps")
                            for k in range(4):
                                nc.tensor.matmul(ps[:, :nw],
                                                 wk_sb[:, k, 128 * dc:128 * dc + 128],
                                                 xkv_st[:, 0, k, off:off + nw],
                                                 start=(k == 0), stop=(k == 3))
                            nc.scalar.copy(kT[:, dc, off:off + nw], ps[:, :nw])

                    # ---- v projection: per block, 96-strided [128, H, 64] ----
                    vts = {}
                    for j in blocks:
                        vloc = (QB * j - HALF) - win_lo
                        ps = proj_ps.tile([128, 512], F32, tag="proj", name="vps")
                        for k in range(4):
                            nc.tensor.matmul(ps[:],
                                             xkv_st[:, 1, k, vloc:vloc + 128],
                                             wv_sb[:, k, :],
                                             start=(k == 0), stop=(k == 3))
                        vt = vring[j % NVR]
                        if j % 2 == 0 or os.environ.get("BK_VSPLIT", "0") != "1":
                            nc.vector.tensor_copy(
                                vt[:, :, 0:64],
                                ps[:].rearrange("p (h d) -> p h d", d=64))
                        else:
                            nc.scalar.copy(
                                vt[:, :, 0:64],
                                ps[:].rearrange("p (h d) -> p h d", d=64))
                        vts[j] = vt

                    # ---- attention blocks ----
                    atc = smp.tile([128, 4, QW], LOW, tag="atc", name="atc")
                    for j in blocks:
                        qw = _qw(j)
                        qloc = QB * (j - j0)
                        kloc = qloc                          # window base in kT
                        pT = ptp.tile([128, H, QB], LOW, tag="pT", name="pT")
                        gps = []
                        EXPM = os.environ.get("BK_EXPM", "0") == "1"
                        st2 = (st_ps.tile([128, 2, 4, 128], F32, tag="st",
                                          name="st") if EXPM else None)
                        for g in range(2):
                            st = (st2[:, g, :, 0:QB] if EXPM else
                                  st_ps.tile([128, 4, QB], F32, tag="st",
                                             name="st"))
                            for u in range(2):
                                hc = 2 * g + u          # kT chunk = head pair
                                if BK_QC:
                                    for hb in range(2):
                                        nc.tensor.matmul(
                                            st[:, 2 * u + hb, :qw],
                                            kT[64 * hb:64 * hb + 64, hc,
                                               kloc:kloc + 128],
                                            qC[64 * hb:64 * hb + 64, hc,
                                               qloc:qloc + qw],
                                            start=True, stop=True,
                                            tile_position=(64 * hb, 0))
                                else:
                                    nc.tensor.matmul(
                                        st[:, 2 * u:2 * u + 2, :qw],
                                        kT[:, hc, kloc:kloc + 128],
                                        qZ[:, 2 * hc:2 * hc + 2, qloc:qloc + qw],
                                        start=True, stop=True)
                            if not EXPM:
                                nc.scalar.activation(
                                    pT[:, 4 * g:4 * g + 4, :qw],
                                    st[:, :, :qw], EXP,
                                    bias=pen[:, j:j + 1], scale=1.0)
                                nc.vector.tensor_tensor(
                                    out=pT[:, 4 * g:4 * g + 4, :qw],
                                    in0=pT[:, 4 * g:4 * g + 4, :qw],
                                    in1=band[:, 4 * g:4 * g + 4, :qw], op=MULT)
                        if EXPM:
                            pTv = pT.rearrange("p (g h) q -> p g h q", g=2)
                            nc.scalar.activation(
                                pTv[:, :, :, :qw], st2[:, :, :, :qw], EXP,
                                bias=pen[:, j:j + 1], scale=1.0)
                            nc.vector.tensor_tensor(
                                out=pT[:, :, :qw], in0=pT[:, :, :qw],
                                in1=band[:, :, :qw], op=MULT)
                        att = smp.tile([QB, D], LOW, tag="att", name="att")
                        attv = att.rearrange("q (h d) -> q h d", d=64)
                        rc = smp.tile([QB, H], F32, tag="rc", name="rc")
                        for g in range(2):
                            m2 = mm2_ps.tile([QB, 4, 66], F32, tag="m2", name="m2")
                            for hi in range(4):
                                h = 4 * g + hi
                                nc.tensor.matmul(m2[:qw, hi, :], pT[:, h, :qw],
                                                 vts[j][:, h, :], start=True, stop=True)
                            gps.append(m2)
                        for g in range(2):
                            nc.vector.reciprocal(rc[:qw, 4 * g:4 * g + 4],
                                                 gps[g][:qw, :, 64])
                        for g in range(2):
                            nc.vector.tensor_tensor(
                                out=attv[:qw, 4 * g:4 * g + 4, :],
                                in0=gps[g][:qw, :, 0:64],
                                in1=rc[:qw, 4 * g:4 * g + 4].unsqueeze(2).to_broadcast(
                                    (qw, 4, 64)),
                                op=MULT)
                        # transpose attn -> [dcat, q], gathered per chunk
                        tr = (mm2_ps.tile([128, 4, QB], LOW, tag="m2",
                                          name="tr")
                              if os.environ.get("BK_EXPM", "0") == "1" else
                              tr_ps.tile([128, 4, QB], LOW, tag="tr",
                                         name="tr"))
                        for i in range(4):
                            nc.tensor.transpose(tr[:, i, :qw],
                                                att[:qw, 128 * i:128 * i + 128],
                                                iden[:qw, :qw])
                        nc.scalar.copy(atc[:, :, qloc:qloc + qw], tr[:, :, :qw])

                    # ---- output projection per 128-query tile (bias on host) ----
                    BATCH_OUT = os.environ.get("BK_OUTB", "1") == "1"
                    OUTB2 = os.environ.get("BK_OUTB2", "1") == "1"
                    TW = (120 if qwid % 120 == 0 else 128) if BATCH_OUT else 128
                    nt = (qwid + TW - 1) // TW
                    pair2 = BATCH_OUT and OUTB2 and qwid == 480
                    if pair2:
                        if ci % 2 == 0:
                            osc2 = outp.tile([120, 8, D], F16, tag="osb",
                                             name="osb")
                            osc2_lo = q_lo
                        osc = osc2
                        toff = 4 * (ci % 2)
                    else:
                        osc = (outp.tile([TW, nt, D], F16, tag="osb", name="osb")
                               if BATCH_OUT else None)
                        toff = 0
                    for t in range(nt):
                        tw = min(TW, qwid - TW * t)
                        op = op_ps.tile([128, D], F32,
                                        tag="proj" if op_ps is proj_ps else "op",
                                        name="op")
                        for i in range(4):
                            nc.tensor.matmul(op[:tw, :],
                                             atc[:, i, TW * t:TW * t + tw],
                                             wo_sb[:, i, :],
                                             start=(i == 0), stop=(i == 3))
                        if BATCH_OUT:
                            nc.scalar.copy(osc[:tw, toff + t, :], op[:tw, :])
                        else:
                            osb = outp.tile([128, D], F16, tag="osb", name="osb")
                            nc.scalar.copy(osb[:tw, :], op[:tw, :])
                            nc.scalar.dma_start(
                                out_d[q_lo + TW * t:q_lo + TW * t + tw, :],
                                osb[:tw, :])
                    if BATCH_OUT and pair2 and ci % 2 == 1:
                        nc.sync.dma_start(
                            out_d[osc2_lo:q_hi, :].rearrange(
                                "(c p) d -> p c d", p=120),
                            osc2[:, :, :])
                    elif BATCH_OUT and not pair2:
                        nc.sync.dma_start(
                            out_d[q_lo:q_hi, :].rearrange("(c p) d -> p c d", p=TW),
                            osc[:, :, :])

    nc.finalize()
    return nc


def _host_consts():
    rr = np.arange(128)[:, None]
    qq = np.arange(QB)[None, :]
    band = (((rr - qq) >= 0) & ((rr - qq) <= 32)).astype(np.float32)
    band8 = np.tile(band, (1, H))
    ident = np.eye(128, dtype=np.float32)
    return band8, ident


def _get_nc(reps=None):
    reps = REPS if reps is None else reps
    key = (reps, USE_BF16)
    if key not in _NCS:
        _NCS[key] = _build(reps)
    return _NCS[key]


def _prep_inmaps(query, key, value, mask, Wq, bq, Wk, bk, Wv, bv, Wo, bo):
    query = np.asarray(query, np.float32)
    key = np.asarray(key, np.float32)
    value = np.asarray(value, np.float32)
    mask = np.asarray(mask)
    Wq, bq = np.asarray(Wq, np.float32), np.asarray(bq, np.float32)
    Wk = np.asarray(Wk, np.float32)
    Wv = np.asarray(Wv, np.float32)
    Wo = np.asarray(Wo, np.float32)

    band8, ident = _host_consts()
    jj = np.arange(NB)[None, :]
    rr = np.arange(128)[:, None]
    pos = QB * jj - HALF + rr                      # [128, NB]
    valid = (pos >= 0) & (pos < S)
    posc = np.clip(pos, 0, S - 1)

    import ml_dtypes
    lowdt = ml_dtypes.bfloat16
    xdt = np.float16
    common = {
        "wqkvT": np.concatenate(
            [np.ascontiguousarray(W.T).astype(xdt) for W in (Wq, Wk, Wv)],
            axis=0),
        "woT": np.ascontiguousarray(Wo.T).astype(lowdt),
        "cst16": np.concatenate(
            [band8, ident], axis=1).astype(lowdt),
    }
    bqc_h = np.ascontiguousarray(bq.reshape(4, 128).T).astype(np.float32)
    from concurrent.futures import ThreadPoolExecutor

    def _one(b):
        pen = np.where(valid & ~mask[b][posc], 0.0, NEG).astype(np.float32)
        cst32 = np.concatenate([bqc_h, pen], axis=1).astype(np.float32)
        if (os.environ.get("BK_XALL", "0") == "1"
                or os.environ.get("BK_RES", "0") == "1"):
            return dict(
                common,
                xallT=np.pad(
                    np.concatenate([query[b].T, key[b].T, value[b].T],
                                   axis=0).astype(xdt),
                    ((0, 0), (HALF, 48))),
                cst32=cst32,
            )
        return dict(
            common,
            xqT=np.ascontiguousarray(query[b].T).astype(xdt),
            xkvT=np.pad(
                np.concatenate([key[b].T, value[b].T], axis=0).astype(xdt),
                ((0, 0), (HALF, 48))),
            cst32=cst32,
        )

    with ThreadPoolExecutor(max_workers=8) as ex:
        in_maps = list(ex.map(_one, range(B)))
    return in_maps


def kernel(**inputs):
    from concourse.bass_utils import run_bass_kernel_spmd
    in_maps = _prep_inmaps(**inputs)
    res = run_bass_kernel_spmd(_get_nc(), in_maps, core_ids=list(range(8)))
    Wo = np.asarray(inputs["Wo"], np.float32)
    bv = np.asarray(inputs["bv"], np.float32)
    bo = np.asarray(inputs["bo"], np.float32)
    boeff = (Wo @ bv + bo).astype(np.float32)      # softmax rows sum to 1
    return np.stack([res.results[c]["out"].astype(np.float32) + boeff
                     for c in range(B)], axis=0)

